# revision 5
# baseline (speedup 1.0000x reference)
"""GCN message-passing kernel for Trainium2 (8 NeuronCores).

Strategy (v2):
  - Nodes sharded across 8 cores, aligned to graph boundaries (G/8 graphs/core).
  - Per layer each core computes z = (dinv * h) @ W for its shard, the z-table
    is AllGathered (bf16), source rows are fetched with dma_gather (4 int16
    sub-tables), and one-hot scatter-matmuls accumulate messages per 128-dst
    block in PSUM.
  - Self-loop messages are NOT gathered: agg += z_local directly (SBUF add),
    which cuts gather rows and lowers per-group maxima.
  - Variable per-(block, tab) KD packing minimizes slot padding (the SWDGE
    descriptor generation on GPSIMD at ~8ns/row is the bottleneck engine).
  - Graph pooling: masked static-window reduces on h^T + one AllGather.
"""

import sys

sys.path.insert(0, "/opt/trn_rl_repo")

import numpy as np
import ml_dtypes

import concourse.bass as bass
import concourse.bacc as bacc
import concourse.tile as tile
from concourse import mybir, library_config
from concourse.bass_utils import run_bass_kernel_spmd

C = 8            # cores
P = 128          # partitions / block size
HID = 128
SBW = 4          # dst blocks per superblock
GCHUNK = 2048    # max idxs per dma_gather call

LAST_RESULTS = None  # set by kernel(): BassKernelResults of the last run
TRACE = False        # set True (e.g. by test.py) to capture an NTFF profile


def _host_prep(x, edge_index, batch, W_emb, b_emb, W_conv, b_conv, W1, b1, W2, b2):
    N = x.shape[0]
    batch = np.asarray(batch, dtype=np.int64)
    G = int(batch.max()) + 1
    assert G % C == 0, G
    L = W_conv.shape[0]

    src = np.asarray(edge_index[0], dtype=np.int64)
    dst = np.asarray(edge_index[1], dtype=np.int64)

    # degree includes the self-loop (reference concatenates self edges)
    deg = (np.bincount(dst, minlength=N) + 1).astype(np.float64)
    dinv = (1.0 / np.sqrt(np.maximum(deg, 1e-12))).astype(np.float32)

    gpc = G // C
    bounds = np.searchsorted(batch, np.arange(G + 1))
    starts = bounds[np.arange(C) * gpc]
    ends = bounds[(np.arange(C) + 1) * gpc]
    shard_sizes = ends - starts
    NB = int(np.ceil(shard_sizes.max() / P))
    SHARD_PAD = NB * P
    TAB_ROWS = 2 * SHARD_PAD
    assert TAB_ROWS <= 32767, TAB_ROWS

    core_of = np.searchsorted(ends - 1, np.arange(N), side="left")
    table_row = core_of * SHARD_PAD + (np.arange(N) - starts[core_of])
    e_row = table_row[src]
    e_tab = (e_row // TAB_ROWS).astype(np.int64)
    e_trow = (e_row % TAB_ROWS).astype(np.int64)
    e_core = core_of[dst]
    e_dloc = dst - starts[e_core]
    dblk = e_dloc // P

    NSB = int(np.ceil(NB / SBW))

    NTAB = 4
    cntG = np.zeros((C, NB, NTAB), dtype=np.int64)
    np.add.at(cntG, (e_core, dblk, e_tab), 1)
    KDG = np.maximum(1, np.ceil(cntG.max(axis=0) / P)).astype(np.int64)  # [NB,NTAB]

    # ---- schedule ----
    # gather stream: (sb, t, d, kb); indirect stream: (sb, d, kb)
    sched = []
    gslot = 0   # gather slot offset
    mG = 0      # one-hot column offset (gather)
    for s in range(NSB):
        blocks = list(range(s * SBW, min((s + 1) * SBW, NB)))
        tabs = []
        contrib = {d: [] for d in blocks}
        for t in range(NTAB):
            seg = int(sum(KDG[d, t] for d in blocks)) * P
            tabs.append(dict(seg=seg, mcol0=mG, gslot0=gslot))
            srel = 0
            for d in blocks:
                for kb in range(int(KDG[d, t])):
                    contrib[d].append(dict(t=t, srel=srel, mcol=mG))
                    srel += P
                    mG += 1
            gslot += seg
        sched.append(dict(blocks=blocks, tabs=tabs, contrib=contrib))
    TOT_G = gslot
    NMM_G = mG

    # ---- per-core slot filling ----
    idxG_cores, dstG_cores = [], []
    for c in range(C):
        mc = e_core == c
        key = (dblk[mc] * NTAB + e_tab[mc])
        order = np.argsort(key, kind="stable")
        trow_s = e_trow[mc][order]
        dloc_s = e_dloc[mc][order]
        key_s = key[order]
        first = np.searchsorted(key_s, np.arange(NB * NTAB))
        cnt_k = np.diff(np.append(first, len(key_s)))

        idx_arr = np.zeros(TOT_G, dtype=np.int16)
        dg_arr = np.full((NMM_G, P), -1.0, dtype=np.float32)
        for sdef in sched:
            for t in range(NTAB):
                tdef = sdef["tabs"][t]
                base = tdef["gslot0"]
                srel = 0
                for d in sdef["blocks"]:
                    k = d * NTAB + t
                    n = cnt_k[k]
                    f = first[k]
                    sl = base + srel
                    idx_arr[sl:sl + n] = trow_s[f:f + n].astype(np.int16)
                    mrow = tdef["mcol0"] + (srel // P)
                    dl = (dloc_s[f:f + n] % P).astype(np.float32)
                    kd = int(KDG[d, t])
                    pad = kd * P - n
                    assert pad >= 0
                    dfull = np.concatenate([dl, np.full(pad, -1.0, np.float32)])
                    dg_arr[mrow:mrow + kd] = dfull.reshape(kd, P)
                    srel += kd * P
        idxG_cores.append(np.tile(
            np.ascontiguousarray(idx_arr.reshape(-1, 16).T), (C, 1)
        ))
        dstG_cores.append(np.ascontiguousarray(
            dg_arr.T.astype(ml_dtypes.bfloat16)
        ))

    # ---- pooling windows ----
    lo_u = np.zeros(gpc, dtype=np.int64)
    wd_u = np.zeros(gpc, dtype=np.int64)
    for g in range(gpc):
        los = bounds[np.arange(C) * gpc + g] - starts
        his = bounds[np.arange(C) * gpc + g + 1] - starts
        lo_u[g] = los.min()
        wd_u[g] = max(his.max() - lo_u[g], 1)
    HT_W = int(max(SHARD_PAD, (lo_u + wd_u).max()))

    gid_cores = []
    for c in range(C):
        gid = np.full(HT_W, -1.0, dtype=np.float32)
        n = shard_sizes[c]
        gid[:n] = (batch[starts[c]:ends[c]] - c * gpc).astype(np.float32)
        gid_cores.append(
            np.ascontiguousarray(np.tile(gid.astype(ml_dtypes.bfloat16), (P, 1)))
        )

    cnt = (bounds[1:] - bounds[:-1]).astype(np.float32)
    cntinv = (1.0 / np.maximum(cnt, 1.0)).astype(np.float32)

    xT_cores, dinv_cores = [], []
    D_IN = x.shape[1]
    for c in range(C):
        xs = np.zeros((SHARD_PAD, D_IN), dtype=np.float32)
        xs[: shard_sizes[c]] = x[starts[c]:ends[c]]
        xT_cores.append(np.ascontiguousarray(xs.T))
        dv = np.zeros(SHARD_PAD, dtype=np.float32)
        dv[: shard_sizes[c]] = dinv[starts[c]:ends[c]]
        dinv_cores.append(np.ascontiguousarray(dv.reshape(NB, P).T))

    cfg = dict(
        N=N, G=G, L=L, gpc=gpc, NB=NB, SHARD_PAD=SHARD_PAD, TAB_ROWS=TAB_ROWS,
        NSB=NSB, NTAB=NTAB, TOT_G=TOT_G, NMM_G=NMM_G,
        D_IN=D_IN, HT_W=HT_W, lo_u=lo_u.tolist(), wd_u=wd_u.tolist(),
        D_OUT=W2.shape[1], H1=W1.shape[1], sched=sched,
    )

    common = dict(
        W_emb=np.asarray(W_emb, np.float32),
        W_conv=np.asarray(W_conv, np.float32).reshape(L * HID, HID),
        W1=np.asarray(W1, np.float32),
        W2=np.asarray(W2, np.float32),
        b_emb_b=np.tile(np.asarray(b_emb, np.float32), (P, 1)),
        b_conv_b=np.tile(
            np.asarray(b_conv, np.float32)[:, None, :], (1, P, 1)
        ).reshape(L * P, HID),
        b1_b=np.tile(np.asarray(b1, np.float32), (P, 1)),
        b2_b=np.tile(np.asarray(b2, np.float32), (P, 1)),
        iota=np.tile(np.arange(P, dtype=np.float32), (P, 1)).astype(
            ml_dtypes.bfloat16
        ),
        ident=np.eye(P, dtype=np.float32),
        cntinv=cntinv.reshape(G, 1),
    )
    per_core = [
        dict(
            xT=xT_cores[c], dinv_t=dinv_cores[c],
            idxg=idxG_cores[c], dstg=dstG_cores[c], gid=gid_cores[c],
        )
        for c in range(C)
    ]
    return cfg, common, per_core


def _build(cfg):
    G, L = cfg["G"], cfg["L"]
    gpc, NB, SHARD_PAD = cfg["gpc"], cfg["NB"], cfg["SHARD_PAD"]
    TAB_ROWS, NSB, NTAB = cfg["TAB_ROWS"], cfg["NSB"], cfg["NTAB"]
    TOT_G, NMM_G = cfg["TOT_G"], cfg["NMM_G"]
    D_IN, HT_W = cfg["D_IN"], cfg["HT_W"]
    D_OUT, H1 = cfg["D_OUT"], cfg["H1"]
    sched = cfg["sched"]
    H1H = H1 // 2
    WDMAX = int(max(cfg["wd_u"]))
    f32, bf16 = mybir.dt.float32, mybir.dt.bfloat16
    i16, i32 = mybir.dt.int16, mybir.dt.int32
    AFT = mybir.ActivationFunctionType
    Alu = mybir.AluOpType

    nc = bacc.Bacc("TRN2", target_bir_lowering=False, debug=False, num_devices=C)

    xT = nc.dram_tensor("xT", [D_IN, SHARD_PAD], f32, kind="ExternalInput")
    W_emb = nc.dram_tensor("W_emb", [D_IN, HID], f32, kind="ExternalInput")
    W_conv = nc.dram_tensor("W_conv", [L * HID, HID], f32, kind="ExternalInput")
    W1 = nc.dram_tensor("W1", [3 * HID, H1], f32, kind="ExternalInput")
    W2 = nc.dram_tensor("W2", [H1, D_OUT], f32, kind="ExternalInput")
    b_emb_b = nc.dram_tensor("b_emb_b", [P, HID], f32, kind="ExternalInput")
    b_conv_b = nc.dram_tensor("b_conv_b", [L * P, HID], f32, kind="ExternalInput")
    b1_b = nc.dram_tensor("b1_b", [P, H1], f32, kind="ExternalInput")
    b2_b = nc.dram_tensor("b2_b", [P, D_OUT], f32, kind="ExternalInput")
    iota_d = nc.dram_tensor("iota", [P, P], bf16, kind="ExternalInput")
    ident_d = nc.dram_tensor("ident", [P, P], f32, kind="ExternalInput")
    cntinv_d = nc.dram_tensor("cntinv", [G, 1], f32, kind="ExternalInput")
    dinv_d = nc.dram_tensor("dinv_t", [P, NB], f32, kind="ExternalInput")
    idxg_d = nc.dram_tensor("idxg", [P, TOT_G // 16], i16, kind="ExternalInput")
    dstg_d = nc.dram_tensor("dstg", [P, NMM_G], bf16, kind="ExternalInput")
    gid_d = nc.dram_tensor("gid", [P, HT_W], bf16, kind="ExternalInput")
    out_d = nc.dram_tensor("out", [G, D_OUT], f32, kind="ExternalOutput")

    z_local = nc.dram_tensor("z_local", [SHARD_PAD, HID], bf16, kind="Internal")
    z_tables = [
        nc.dram_tensor(f"z_table{i}", [C * SHARD_PAD, HID], bf16, kind="Internal")
        for i in range(2)
    ]
    pool_loc = nc.dram_tensor("pool_loc", [gpc, 2 * HID], f32, kind="Internal")
    pool_all = nc.dram_tensor("pool_all", [G, 2 * HID], f32, kind="Internal")

    rg = [list(range(C))]
    ZBW = max(SHARD_PAD, HT_W)

    with tile.TileContext(nc) as tc:
        with (
            tc.tile_pool(name="const", bufs=1) as cpool,
            tc.tile_pool(name="big", bufs=1) as bigpool,
            tc.tile_pool(name="g", bufs=3) as gpool,
            tc.tile_pool(name="s", bufs=2) as spool,
            tc.tile_pool(name="work", bufs=2) as wpool,
            tc.tile_pool(name="zst", bufs=2) as zpool,
            tc.tile_pool(name="ps", bufs=3, space="PSUM") as pspool,
            tc.tile_pool(name="agg", bufs=4, space="PSUM") as aggpool,
        ):
            nc.gpsimd.load_library(library_config.mlp)

            def cload(dram_ap, shape, dtype, nm):
                t = cpool.tile(shape, dtype, name=nm, tag=nm)
                nc.sync.dma_start(t[:], dram_ap)
                return t

            Wemb_s = cload(W_emb[:], [D_IN, HID], f32, "Wemb_s")
            Wc_s = cload(
                W_conv[:].rearrange("(l k) h -> k l h", k=P), [P, L, HID], f32
            , "Wc_s")
            W1_s = cload(W1[:].rearrange("(a k) h -> k a h", k=P), [P, 3, H1], f32, "W1_s")
            W2a_s = cload(W2[0:H1H, :], [H1H, D_OUT], f32, "W2a_s")
            W2b_s = cload(W2[H1H:H1, :], [H1H, D_OUT], f32, "W2b_s")
            bemb_s = cload(b_emb_b[:], [P, HID], f32, "bemb_s")
            bconv_s = cload(
                b_conv_b[:].rearrange("(l k) h -> k l h", k=P), [P, L, HID], f32
            , "bconv_s")
            b1_s = cload(b1_b[:], [P, H1], f32, "b1_s")
            b2_s = cload(b2_b[:], [P, D_OUT], f32, "b2_s")
            iota_s = cload(iota_d[:], [P, P], bf16, "iota_s")
            ident_s = cload(ident_d[:], [P, P], f32, "ident_s")
            cnt_s = cload(cntinv_d[:], [G, 1], f32, "cnt_s")
            dinv_s = cload(dinv_d[:], [P, NB], f32, "dinv_s")
            dstg_s = cload(dstg_d[:], [P, NMM_G], bf16, "dstg_s")
            idxg_s = cload(idxg_d[:], [P, TOT_G // 16], i16, "idxg_s")

            hbuf = bigpool.tile([P, SHARD_PAD], f32, tag="h")
            zbuf = bigpool.tile([P, ZBW], bf16, tag="zb")

            # ---- embed: h'0 = dinv * (x @ W_emb + b_emb) ----
            for b4 in range(0, NB, 4):
                nb4 = min(4, NB - b4)
                ps = pspool.tile([P, 4 * HID], f32, tag="ps")
                for j in range(nb4):
                    b = b4 + j
                    xt_b = wpool.tile([D_IN, P], f32, tag="xt")
                    nc.sync.dma_start(xt_b[:], xT[:, b * P:(b + 1) * P])
                    nc.tensor.matmul(
                        ps[:, j * HID:(j + 1) * HID],
                        lhsT=xt_b[:], rhs=Wemb_s[:],
                        start=True, stop=True,
                    )
                nc.vector.tensor_copy(
                    hbuf[:, b4 * P: b4 * P + nb4 * HID], ps[:, : nb4 * HID]
                )
            nc.vector.tensor_tensor(
                out=hbuf[:].rearrange("p (b h) -> p b h", h=HID),
                in0=hbuf[:].rearrange("p (b h) -> p b h", h=HID),
                in1=bemb_s[:].rearrange("p (a h) -> p a h", a=1).to_broadcast(
                    [P, NB, HID]
                ),
                op=Alu.add,
            )
            nc.vector.tensor_tensor(
                out=hbuf[:].rearrange("p (b k) -> p b k", k=P),
                in0=hbuf[:].rearrange("p (b k) -> p b k", k=P),
                in1=dinv_s[:].to_broadcast([P, NB, P]),
                op=Alu.mult,
            )

            # ---- layers ----
            for l in range(L):
                zt = z_tables[l % 2]
                # z compute (node-major) + zbuf copy + z_local write
                for b8 in range(0, NB, SBW):
                    nb8 = min(SBW, NB - b8)
                    zstage = zpool.tile([P, SBW * HID], bf16, tag="zst")
                    for j in range(nb8):
                        b = b8 + j
                        pst = pspool.tile([P, P], f32, tag="ps")
                        nc.tensor.transpose(
                            out=pst[:], in_=hbuf[:, b * P:(b + 1) * P],
                            identity=ident_s[:],
                        )
                        hT_b = wpool.tile([P, P], f32, tag="hTb")
                        nc.vector.tensor_copy(hT_b[:], pst[:])
                        psz = pspool.tile([P, HID], f32, tag="ps")
                        nc.tensor.matmul(
                            psz[:], lhsT=hT_b[:], rhs=Wc_s[:, l, :],
                            start=True, stop=True,
                        )
                        nc.vector.tensor_copy(
                            zstage[:, j * HID:(j + 1) * HID], psz[:]
                        )
                    nc.scalar.activation(
                        zbuf[:, b8 * HID: b8 * HID + nb8 * HID],
                        zstage[:, : nb8 * HID], AFT.Copy,
                    )
                    nc.sync.dma_start(
                        z_local[b8 * P: b8 * P + nb8 * P, :].rearrange(
                            "(b p) h -> p b h", p=P
                        ),
                        zstage[:, : nb8 * HID].rearrange(
                            "p (b h) -> p b h", h=HID
                        ),
                    )
                nc.gpsimd.collective_compute(
                    "AllGather", Alu.bypass,
                    replica_groups=rg,
                    ins=[z_local[:].opt()], outs=[zt[:].opt()],
                )

                for sdef in sched:
                    blocks = sdef["blocks"]
                    aggs = {
                        d: aggpool.tile([P, HID], f32, tag="agg",
                                        name=f"agg_{l}_{d}")
                        for d in blocks
                    }
                    # gather path per sub-table
                    gtile_map = {}
                    for t in range(NTAB):
                        tdef = sdef["tabs"][t]
                        seg = tdef["seg"]
                        gtiles = []
                        off = 0
                        while off < seg:
                            n = min(GCHUNK, seg - off)
                            gt = gpool.tile([P, GCHUNK // P, HID], bf16,
                                            tag="g")
                            s0 = tdef["gslot0"] + off
                            nc.gpsimd.dma_gather(
                                gt[:, : n // P, :],
                                zt[t * TAB_ROWS:(t + 1) * TAB_ROWS, :],
                                idxg_s[:, s0 // 16:(s0 + n) // 16],
                                n, n, HID, single_packet=False,
                            )
                            gtiles.append(gt)
                            off += n
                        gtile_map[t] = gtiles
                        sbt = spool.tile([P, seg], bf16, tag="sg",
                                         name=f"sbt_{l}_{tdef['mcol0']}")
                        nc.vector.tensor_tensor(
                            out=sbt[:].rearrange("p (a k) -> p a k", k=P),
                            in0=dstg_s[
                                :, tdef["mcol0"]:tdef["mcol0"] + seg // P
                            ].to_broadcast([P, seg // P, P]),
                            in1=iota_s[:].rearrange(
                                "p (a k) -> p a k", a=1
                            ).to_broadcast([P, seg // P, P]),
                            op=Alu.is_equal,
                        )
                        tdef["sbt"] = sbt

                    # scatter matmuls
                    for d in blocks:
                        cons = sdef["contrib"][d]
                        for ci, con in enumerate(cons):
                            tdef = sdef["tabs"][con["t"]]
                            srel = con["srel"]
                            gi_, col = srel // GCHUNK, (srel % GCHUNK) // P
                            sb_col = con["mcol"] - tdef["mcol0"]
                            nc.tensor.matmul(
                                aggs[d][:],
                                lhsT=tdef["sbt"][
                                    :, sb_col * P:(sb_col + 1) * P
                                ],
                                rhs=gtile_map[con["t"]][gi_][:, col, :],
                                start=(ci == 0), stop=(ci == len(cons) - 1),
                            )
                    for d in blocks:
                        nc.vector.tensor_copy(
                            hbuf[:, d * P:(d + 1) * P], aggs[d][:]
                        )

                # epilogue: h = tanh(dinv*(agg + z_self) + b); premult dinv
                nc.vector.tensor_tensor(
                    out=hbuf[:].rearrange("p (b h) -> p b h", h=HID),
                    in0=hbuf[:].rearrange("p (b h) -> p b h", h=HID),
                    in1=zbuf[:, :SHARD_PAD].rearrange(
                        "p (b h) -> p b h", h=HID
                    ),
                    op=Alu.add,
                )
                nc.vector.tensor_tensor(
                    out=hbuf[:].rearrange("p (b k) -> p b k", k=P),
                    in0=hbuf[:].rearrange("p (b k) -> p b k", k=P),
                    in1=dinv_s[:].to_broadcast([P, NB, P]),
                    op=Alu.mult,
                )
                nc.vector.tensor_tensor(
                    out=hbuf[:].rearrange("p (b h) -> p b h", h=HID),
                    in0=hbuf[:].rearrange("p (b h) -> p b h", h=HID),
                    in1=bconv_s[:, l, :].rearrange(
                        "p (a h) -> p a h", a=1
                    ).to_broadcast([P, NB, HID]),
                    op=Alu.add,
                )
                nc.scalar.activation(hbuf[:], hbuf[:], AFT.Tanh)
                if l < L - 1:
                    nc.vector.tensor_tensor(
                        out=hbuf[:].rearrange("p (b k) -> p b k", k=P),
                        in0=hbuf[:].rearrange("p (b k) -> p b k", k=P),
                        in1=dinv_s[:].to_broadcast([P, NB, P]),
                        op=Alu.mult,
                    )

            # ---- pooling ----
            hT = bigpool.tile([P, HT_W], bf16, tag="hT")
            if HT_W > SHARD_PAD:
                nc.vector.memset(hT[:, SHARD_PAD:], 0.0)
            for b in range(NB):
                pst = pspool.tile([P, P], f32, tag="ps")
                nc.tensor.transpose(
                    out=pst[:], in_=hbuf[:, b * P:(b + 1) * P],
                    identity=ident_s[:],
                )
                nc.vector.tensor_copy(hT[:, b * P:(b + 1) * P], pst[:])
            gid_s = bigpool.tile([P, ZBW], bf16, tag="zb")
            nc.sync.dma_start(gid_s[:, :HT_W], gid_d[:])

            sumP = wpool.tile([P, gpc], f32, tag="sumP")
            maxP = wpool.tile([P, gpc], f32, tag="maxP")
            for g in range(gpc):
                lo, wd = cfg["lo_u"][g], cfg["wd_u"][g]
                eq = wpool.tile([P, WDMAX], bf16, tag="eq")
                nc.vector.tensor_scalar(
                    eq[:, :wd], gid_s[:, lo:lo + wd], float(g), None,
                    Alu.is_equal,
                )
                msk = wpool.tile([P, WDMAX], f32, tag="msk")
                nc.vector.tensor_tensor(
                    out=msk[:, :wd], in0=hT[:, lo:lo + wd], in1=eq[:, :wd],
                    op=Alu.mult,
                )
                nc.vector.reduce_sum(
                    sumP[:, g:g + 1], msk[:, :wd], axis=mybir.AxisListType.X
                )
                nc.vector.tensor_scalar(
                    msk[:, :wd], eq[:, :wd], 60.0, -60.0, Alu.mult, Alu.add
                )
                nc.vector.tensor_tensor(
                    out=msk[:, :wd], in0=hT[:, lo:lo + wd], in1=msk[:, :wd],
                    op=Alu.add,
                )
                nc.vector.reduce_max(
                    maxP[:, g:g + 1], msk[:, :wd], axis=mybir.AxisListType.X
                )
            pg = pspool.tile([P, 2 * HID], f32, tag="ps")
            nc.tensor.transpose(
                out=pg[:gpc, :HID], in_=sumP[:], identity=ident_s[:]
            )
            nc.tensor.transpose(
                out=pg[:gpc, HID:], in_=maxP[:], identity=ident_s[:]
            )
            pl = wpool.tile([gpc, 2 * HID], f32, tag="pl")
            nc.vector.tensor_copy(pl[:], pg[:gpc, :])
            nc.sync.dma_start(pool_loc[:], pl[:])
            nc.gpsimd.collective_compute(
                "AllGather", Alu.bypass, replica_groups=rg,
                ins=[pool_loc[:].opt()], outs=[pool_all[:].opt()],
            )
            pa = wpool.tile([G, 2 * HID], f32, tag="pa")
            nc.sync.dma_start(pa[:], pool_all[:])
            mfix = wpool.tile([G, HID], f32, tag="mfix")
            nc.vector.tensor_scalar(
                mfix[:], pa[:, HID:], -50.0, None, Alu.is_gt
            )
            nc.vector.tensor_tensor(
                out=pa[:, HID:], in0=pa[:, HID:], in1=mfix[:], op=Alu.mult
            )
            mean_gf = wpool.tile([G, HID], f32, tag="mean")
            nc.vector.tensor_scalar(
                mean_gf[:], pa[:, :HID], cnt_s[:, :1], None, Alu.mult
            )
            gT = wpool.tile([P, 3 * G], f32, tag="gT")
            for a, src_ap in enumerate([pa[:, :HID], pa[:, HID:], mean_gf[:]]):
                ptx = pspool.tile([P, G], f32, tag="ps")
                nc.tensor.transpose(
                    out=ptx[:, :G], in_=src_ap, identity=ident_s[:G, :G]
                )
                nc.vector.tensor_copy(gT[:, a * G:(a + 1) * G], ptx[:, :G])

            # ---- head ----
            ph1 = pspool.tile([G, H1], f32, tag="ps")
            for a in range(3):
                nc.tensor.matmul(
                    ph1[:], lhsT=gT[:, a * G:(a + 1) * G], rhs=W1_s[:, a, :],
                    start=(a == 0), stop=(a == 2),
                )
            g1 = wpool.tile([G, H1], f32, tag="g1")
            nc.vector.tensor_tensor(
                out=g1[:], in0=ph1[:], in1=b1_s[:G, :], op=Alu.add
            )
            nc.scalar.activation(g1[:], g1[:], AFT.Lrelu, alpha=0.01)
            g1T = wpool.tile([H1H, 2 * G], f32, tag="g1T")
            for a in range(2):
                ptt = pspool.tile([H1H, G], f32, tag="ps")
                nc.tensor.transpose(
                    out=ptt[:], in_=g1[:, a * H1H:(a + 1) * H1H],
                    identity=ident_s[:G, :G],
                )
                nc.vector.tensor_copy(g1T[:, a * G:(a + 1) * G], ptt[:])
            ph2 = pspool.tile([G, D_OUT], f32, tag="ps")
            for a in range(2):
                nc.tensor.matmul(
                    ph2[:], lhsT=g1T[:, a * G:(a + 1) * G],
                    rhs=(W2a_s if a == 0 else W2b_s)[:],
                    start=(a == 0), stop=(a == 1),
                )
            go = wpool.tile([G, D_OUT], f32, tag="go")
            nc.vector.tensor_tensor(
                out=go[:], in0=ph2[:], in1=b2_s[:G, :], op=Alu.add
            )
            nc.scalar.activation(go[:], go[:], AFT.Lrelu, alpha=0.01)
            nc.sync.dma_start(out_d[:], go[:])

    nc.compile()
    return nc


def _install_ntff_shim():
    """Provide antenv.axon_hooks (missing in this image) so that
    run_bass_kernel_spmd(trace=True) can capture an NTFF profile via the
    injected libaxon_pjrt.so. Only used when TRACE=True."""
    import types
    import ctypes
    import contextlib

    try:
        from antenv.axon_hooks import get_axon_ntff_profile_hook  # noqa: F401
        return
    except ImportError:
        pass
    so_path = "/opt/axon/libaxon_pjrt.so"
    try:
        lib = ctypes.CDLL(so_path)
    except OSError:
        return
    if not hasattr(lib, "axon_start_nrt_profile"):
        return
    lib.axon_start_nrt_profile.argtypes = [
        ctypes.POINTER(ctypes.c_int64), ctypes.c_size_t,
    ]
    lib.axon_start_nrt_profile.restype = ctypes.c_int64
    lib.axon_stop_nrt_profile.argtypes = [ctypes.c_char_p]
    lib.axon_stop_nrt_profile.restype = ctypes.c_int64

    @contextlib.contextmanager
    def _hook(output_dir, device_ids):
        import jax
        jax.devices()
        if device_ids:
            ids = (ctypes.c_int64 * len(device_ids))(*device_ids)
            rc = lib.axon_start_nrt_profile(ids, len(device_ids))
        else:
            rc = lib.axon_start_nrt_profile(None, 0)
        if rc != 0:
            raise RuntimeError(f"axon_start_nrt_profile rc={rc}")
        try:
            yield
        finally:
            n = lib.axon_stop_nrt_profile(str(output_dir).encode())
            print(f"profile: {n} file(s) written to {output_dir}",
                  file=sys.stderr)

    mod = types.ModuleType("antenv.axon_hooks")
    mod.get_axon_ntff_profile_hook = lambda: _hook
    mod.set_axon_ntff_profile_hook = lambda h: None
    sys.modules["antenv.axon_hooks"] = mod


def kernel(**inputs):
    global LAST_RESULTS
    if TRACE:
        _install_ntff_shim()
    ins = {k: np.asarray(v) for k, v in inputs.items()}
    cfg, common, per_core = _host_prep(
        ins["x"].astype(np.float32), ins["edge_index"], ins["batch"],
        ins["W_emb"], ins["b_emb"], ins["W_conv"], ins["b_conv"],
        ins["W1"], ins["b1"], ins["W2"], ins["b2"],
    )
    nc = _build(cfg)

    in_maps = []
    for c in range(C):
        m = dict(
            xT=per_core[c]["xT"],
            W_emb=common["W_emb"], W_conv=common["W_conv"],
            W1=common["W1"], W2=common["W2"],
            b_emb_b=common["b_emb_b"], b_conv_b=common["b_conv_b"],
            b1_b=common["b1_b"], b2_b=common["b2_b"],
            iota=common["iota"], ident=common["ident"],
            cntinv=common["cntinv"],
            dinv_t=per_core[c]["dinv_t"],
            idxg=per_core[c]["idxg"], dstg=per_core[c]["dstg"],
            gid=per_core[c]["gid"],
        )
        in_maps.append(m)

    res = run_bass_kernel_spmd(
        nc, in_maps, core_ids=list(range(C)), trace=TRACE
    )
    LAST_RESULTS = res
    return np.asarray(res.results[0]["out"], dtype=np.float32)


# revision 6
# speedup vs baseline: 1.1776x; 1.1776x over previous
"""GCN message-passing kernel for Trainium2 (8 NeuronCores).

Strategy (v2):
  - Nodes sharded across 8 cores, aligned to graph boundaries (G/8 graphs/core).
  - Per layer each core computes z = (dinv * h) @ W for its shard, the z-table
    is AllGathered (bf16), source rows are fetched with dma_gather (4 int16
    sub-tables), and one-hot scatter-matmuls accumulate messages per 128-dst
    block in PSUM.
  - Self-loop messages are NOT gathered: agg += z_local directly (SBUF add),
    which cuts gather rows and lowers per-group maxima.
  - Variable per-(block, tab) KD packing minimizes slot padding (the SWDGE
    descriptor generation on GPSIMD at ~8ns/row is the bottleneck engine).
  - Graph pooling: masked static-window reduces on h^T + one AllGather.
"""

import sys

sys.path.insert(0, "/opt/trn_rl_repo")

import numpy as np
import ml_dtypes

import concourse.bass as bass
import concourse.bacc as bacc
import concourse.tile as tile
from concourse import mybir, library_config
from concourse.bass_utils import run_bass_kernel_spmd

C = 8            # cores
P = 128          # partitions / block size
HID = 128
SBW = 4          # dst blocks per superblock
GCHUNK = 2048    # max idxs per dma_gather call

LAST_RESULTS = None  # set by kernel(): BassKernelResults of the last run
TRACE = False        # set True (e.g. by test.py) to capture an NTFF profile


def _host_prep(x, edge_index, batch, W_emb, b_emb, W_conv, b_conv, W1, b1, W2, b2):
    N = x.shape[0]
    batch = np.asarray(batch, dtype=np.int64)
    G = int(batch.max()) + 1
    assert G % C == 0, G
    L = W_conv.shape[0]

    src = np.asarray(edge_index[0], dtype=np.int64)
    dst = np.asarray(edge_index[1], dtype=np.int64)

    # degree includes the self-loop (reference concatenates self edges)
    deg = (np.bincount(dst, minlength=N) + 1).astype(np.float64)
    dinv = (1.0 / np.sqrt(np.maximum(deg, 1e-12))).astype(np.float32)

    gpc = G // C
    bounds = np.searchsorted(batch, np.arange(G + 1))
    starts = bounds[np.arange(C) * gpc]
    ends = bounds[(np.arange(C) + 1) * gpc]
    shard_sizes = ends - starts
    NB = int(np.ceil(shard_sizes.max() / P))
    SHARD_PAD = NB * P
    TAB_ROWS = 2 * SHARD_PAD
    assert TAB_ROWS <= 32767, TAB_ROWS

    core_of = np.searchsorted(ends - 1, np.arange(N), side="left")
    table_row = core_of * SHARD_PAD + (np.arange(N) - starts[core_of])
    e_row = table_row[src]
    e_tab = (e_row // TAB_ROWS).astype(np.int64)
    e_trow = (e_row % TAB_ROWS).astype(np.int64)
    e_core = core_of[dst]
    e_dloc = dst - starts[e_core]
    dblk = e_dloc // P

    NSB = int(np.ceil(NB / SBW))

    NTAB = 4
    cntG = np.zeros((C, NB, NTAB), dtype=np.int64)
    np.add.at(cntG, (e_core, dblk, e_tab), 1)
    KDG = np.maximum(1, np.ceil(cntG.max(axis=0) / P)).astype(np.int64)  # [NB,NTAB]

    # ---- schedule ----
    # gather stream: (sb, t, d, kb); indirect stream: (sb, d, kb)
    sched = []
    gslot = 0   # gather slot offset
    mG = 0      # one-hot column offset (gather)
    for s in range(NSB):
        blocks = list(range(s * SBW, min((s + 1) * SBW, NB)))
        tabs = []
        contrib = {d: [] for d in blocks}
        for t in range(NTAB):
            seg = int(sum(KDG[d, t] for d in blocks)) * P
            tabs.append(dict(seg=seg, mcol0=mG, gslot0=gslot))
            srel = 0
            for d in blocks:
                for kb in range(int(KDG[d, t])):
                    contrib[d].append(dict(t=t, srel=srel, mcol=mG))
                    srel += P
                    mG += 1
            gslot += seg
        sched.append(dict(blocks=blocks, tabs=tabs, contrib=contrib))
    TOT_G = gslot
    NMM_G = mG

    # ---- per-core slot filling ----
    idxG_cores, dstG_cores = [], []
    for c in range(C):
        mc = e_core == c
        key = (dblk[mc] * NTAB + e_tab[mc])
        order = np.argsort(key, kind="stable")
        trow_s = e_trow[mc][order]
        dloc_s = e_dloc[mc][order]
        key_s = key[order]
        first = np.searchsorted(key_s, np.arange(NB * NTAB))
        cnt_k = np.diff(np.append(first, len(key_s)))

        idx_arr = np.zeros(TOT_G, dtype=np.int16)
        dg_arr = np.full((NMM_G, P), -1.0, dtype=np.float32)
        for sdef in sched:
            for t in range(NTAB):
                tdef = sdef["tabs"][t]
                base = tdef["gslot0"]
                srel = 0
                for d in sdef["blocks"]:
                    k = d * NTAB + t
                    n = cnt_k[k]
                    f = first[k]
                    sl = base + srel
                    idx_arr[sl:sl + n] = trow_s[f:f + n].astype(np.int16)
                    mrow = tdef["mcol0"] + (srel // P)
                    dl = (dloc_s[f:f + n] % P).astype(np.float32)
                    kd = int(KDG[d, t])
                    pad = kd * P - n
                    assert pad >= 0
                    dfull = np.concatenate([dl, np.full(pad, -1.0, np.float32)])
                    dg_arr[mrow:mrow + kd] = dfull.reshape(kd, P)
                    srel += kd * P
        idxG_cores.append(np.tile(
            np.ascontiguousarray(idx_arr.reshape(-1, 16).T), (C, 1)
        ))
        dstG_cores.append(np.ascontiguousarray(
            dg_arr.T.astype(ml_dtypes.bfloat16)
        ))

    # ---- pooling windows ----
    lo_u = np.zeros(gpc, dtype=np.int64)
    wd_u = np.zeros(gpc, dtype=np.int64)
    for g in range(gpc):
        los = bounds[np.arange(C) * gpc + g] - starts
        his = bounds[np.arange(C) * gpc + g + 1] - starts
        lo_u[g] = los.min()
        wd_u[g] = max(his.max() - lo_u[g], 1)
    HT_W = int(max(SHARD_PAD, (lo_u + wd_u).max()))

    gid_cores = []
    for c in range(C):
        gid = np.full(HT_W, -1.0, dtype=np.float32)
        n = shard_sizes[c]
        gid[:n] = (batch[starts[c]:ends[c]] - c * gpc).astype(np.float32)
        gid_cores.append(
            np.ascontiguousarray(np.tile(gid.astype(ml_dtypes.bfloat16), (P, 1)))
        )

    cnt = (bounds[1:] - bounds[:-1]).astype(np.float32)
    cntinv = (1.0 / np.maximum(cnt, 1.0)).astype(np.float32)

    xT_cores, dinv_cores = [], []
    D_IN = x.shape[1]
    for c in range(C):
        xs = np.zeros((SHARD_PAD, D_IN), dtype=np.float32)
        xs[: shard_sizes[c]] = x[starts[c]:ends[c]]
        xT_cores.append(np.ascontiguousarray(xs.T))
        dv = np.zeros(SHARD_PAD, dtype=np.float32)
        dv[: shard_sizes[c]] = dinv[starts[c]:ends[c]]
        dinv_cores.append(np.ascontiguousarray(dv.reshape(NB, P).T))

    cfg = dict(
        N=N, G=G, L=L, gpc=gpc, NB=NB, SHARD_PAD=SHARD_PAD, TAB_ROWS=TAB_ROWS,
        NSB=NSB, NTAB=NTAB, TOT_G=TOT_G, NMM_G=NMM_G,
        D_IN=D_IN, HT_W=HT_W, lo_u=lo_u.tolist(), wd_u=wd_u.tolist(),
        D_OUT=W2.shape[1], H1=W1.shape[1], sched=sched,
    )

    common = dict(
        W_emb=np.asarray(W_emb, np.float32),
        W_conv=np.asarray(W_conv, np.float32).reshape(L * HID, HID),
        W1=np.asarray(W1, np.float32),
        W2=np.asarray(W2, np.float32),
        b_emb_b=np.tile(np.asarray(b_emb, np.float32), (P, 1)),
        b_conv_b=np.tile(
            np.asarray(b_conv, np.float32)[:, None, :], (1, P, 1)
        ).reshape(L * P, HID),
        b1_b=np.tile(np.asarray(b1, np.float32), (P, 1)),
        b2_b=np.tile(np.asarray(b2, np.float32), (P, 1)),
        iota=np.tile(np.arange(P, dtype=np.float32), (P, 1)).astype(
            ml_dtypes.bfloat16
        ),
        ident=np.eye(P, dtype=np.float32),
        cntinv=cntinv.reshape(G, 1),
    )
    per_core = [
        dict(
            xT=xT_cores[c], dinv_t=dinv_cores[c],
            idxg=idxG_cores[c], dstg=dstG_cores[c], gid=gid_cores[c],
        )
        for c in range(C)
    ]
    return cfg, common, per_core


def _build(cfg):
    G, L = cfg["G"], cfg["L"]
    gpc, NB, SHARD_PAD = cfg["gpc"], cfg["NB"], cfg["SHARD_PAD"]
    TAB_ROWS, NSB, NTAB = cfg["TAB_ROWS"], cfg["NSB"], cfg["NTAB"]
    TOT_G, NMM_G = cfg["TOT_G"], cfg["NMM_G"]
    D_IN, HT_W = cfg["D_IN"], cfg["HT_W"]
    D_OUT, H1 = cfg["D_OUT"], cfg["H1"]
    sched = cfg["sched"]
    H1H = H1 // 2
    WDMAX = int(max(cfg["wd_u"]))
    f32, bf16 = mybir.dt.float32, mybir.dt.bfloat16
    i16, i32 = mybir.dt.int16, mybir.dt.int32
    AFT = mybir.ActivationFunctionType
    Alu = mybir.AluOpType

    nc = bacc.Bacc("TRN2", target_bir_lowering=False, debug=False, num_devices=C)

    xT = nc.dram_tensor("xT", [D_IN, SHARD_PAD], f32, kind="ExternalInput")
    W_emb = nc.dram_tensor("W_emb", [D_IN, HID], f32, kind="ExternalInput")
    W_conv = nc.dram_tensor("W_conv", [L * HID, HID], f32, kind="ExternalInput")
    W1 = nc.dram_tensor("W1", [3 * HID, H1], f32, kind="ExternalInput")
    W2 = nc.dram_tensor("W2", [H1, D_OUT], f32, kind="ExternalInput")
    b_emb_b = nc.dram_tensor("b_emb_b", [P, HID], f32, kind="ExternalInput")
    b_conv_b = nc.dram_tensor("b_conv_b", [L * P, HID], f32, kind="ExternalInput")
    b1_b = nc.dram_tensor("b1_b", [P, H1], f32, kind="ExternalInput")
    b2_b = nc.dram_tensor("b2_b", [P, D_OUT], f32, kind="ExternalInput")
    iota_d = nc.dram_tensor("iota", [P, P], bf16, kind="ExternalInput")
    ident_d = nc.dram_tensor("ident", [P, P], f32, kind="ExternalInput")
    cntinv_d = nc.dram_tensor("cntinv", [G, 1], f32, kind="ExternalInput")
    dinv_d = nc.dram_tensor("dinv_t", [P, NB], f32, kind="ExternalInput")
    idxg_d = nc.dram_tensor("idxg", [P, TOT_G // 16], i16, kind="ExternalInput")
    dstg_d = nc.dram_tensor("dstg", [P, NMM_G], bf16, kind="ExternalInput")
    gid_d = nc.dram_tensor("gid", [P, HT_W], bf16, kind="ExternalInput")
    out_d = nc.dram_tensor("out", [G, D_OUT], f32, kind="ExternalOutput")

    z_local = nc.dram_tensor("z_local", [SHARD_PAD, HID], bf16, kind="Internal")
    z_tables = [
        nc.dram_tensor(f"z_table{i}", [C * SHARD_PAD, HID], bf16, kind="Internal")
        for i in range(2)
    ]
    pool_loc = nc.dram_tensor("pool_loc", [gpc, 2 * HID], f32, kind="Internal")
    pool_all = nc.dram_tensor("pool_all", [G, 2 * HID], f32, kind="Internal")

    rg = [list(range(C))]
    ZBW = max(SHARD_PAD, HT_W)

    with tile.TileContext(nc) as tc:
        with (
            tc.tile_pool(name="const", bufs=1) as cpool,
            tc.tile_pool(name="big", bufs=1) as bigpool,
            tc.tile_pool(name="g", bufs=3) as gpool,
            tc.tile_pool(name="s", bufs=2) as spool,
            tc.tile_pool(name="work", bufs=2) as wpool,
            tc.tile_pool(name="zst", bufs=2) as zpool,
            tc.tile_pool(name="ps", bufs=3, space="PSUM") as pspool,
            tc.tile_pool(name="agg", bufs=4, space="PSUM") as aggpool,
        ):
            nc.gpsimd.load_library(library_config.mlp)

            def cload(dram_ap, shape, dtype, nm):
                t = cpool.tile(shape, dtype, name=nm, tag=nm)
                nc.sync.dma_start(t[:], dram_ap)
                return t

            Wemb_s = cload(W_emb[:], [D_IN, HID], f32, "Wemb_s")
            Wc_s = cload(
                W_conv[:].rearrange("(l k) h -> k l h", k=P), [P, L, HID], f32
            , "Wc_s")
            W1_s = cload(W1[:].rearrange("(a k) h -> k a h", k=P), [P, 3, H1], f32, "W1_s")
            W2a_s = cload(W2[0:H1H, :], [H1H, D_OUT], f32, "W2a_s")
            W2b_s = cload(W2[H1H:H1, :], [H1H, D_OUT], f32, "W2b_s")
            bemb_s = cload(b_emb_b[:], [P, HID], f32, "bemb_s")
            bconv_s = cload(
                b_conv_b[:].rearrange("(l k) h -> k l h", k=P), [P, L, HID], f32
            , "bconv_s")
            b1_s = cload(b1_b[:], [P, H1], f32, "b1_s")
            b2_s = cload(b2_b[:], [P, D_OUT], f32, "b2_s")
            iota_s = cload(iota_d[:], [P, P], bf16, "iota_s")
            ident_s = cload(ident_d[:], [P, P], f32, "ident_s")
            cnt_s = cload(cntinv_d[:], [G, 1], f32, "cnt_s")
            dinv_s = cload(dinv_d[:], [P, NB], f32, "dinv_s")
            dstg_s = cload(dstg_d[:], [P, NMM_G], bf16, "dstg_s")
            idxg_s = cload(idxg_d[:], [P, TOT_G // 16], i16, "idxg_s")

            hbuf = bigpool.tile([P, SHARD_PAD], f32, tag="h")
            zbuf = bigpool.tile([P, ZBW], bf16, tag="zb")

            # ---- embed: h'0 = dinv * (x @ W_emb + b_emb) ----
            for b4 in range(0, NB, 4):
                nb4 = min(4, NB - b4)
                ps = pspool.tile([P, 4 * HID], f32, tag="ps")
                for j in range(nb4):
                    b = b4 + j
                    xt_b = wpool.tile([D_IN, P], f32, tag="xt")
                    nc.sync.dma_start(xt_b[:], xT[:, b * P:(b + 1) * P])
                    nc.tensor.matmul(
                        ps[:, j * HID:(j + 1) * HID],
                        lhsT=xt_b[:], rhs=Wemb_s[:],
                        start=True, stop=True,
                    )
                nc.vector.tensor_copy(
                    hbuf[:, b4 * P: b4 * P + nb4 * HID], ps[:, : nb4 * HID]
                )
            nc.vector.tensor_tensor(
                out=hbuf[:].rearrange("p (b h) -> p b h", h=HID),
                in0=hbuf[:].rearrange("p (b h) -> p b h", h=HID),
                in1=bemb_s[:].rearrange("p (a h) -> p a h", a=1).to_broadcast(
                    [P, NB, HID]
                ),
                op=Alu.add,
            )
            nc.vector.tensor_tensor(
                out=hbuf[:].rearrange("p (b k) -> p b k", k=P),
                in0=hbuf[:].rearrange("p (b k) -> p b k", k=P),
                in1=dinv_s[:].to_broadcast([P, NB, P]),
                op=Alu.mult,
            )

            # ---- layers ----
            for l in range(L):
                zt = z_tables[l % 2]
                # z compute (node-major) + zbuf copy + z_local write
                for b8 in range(0, NB, SBW):
                    nb8 = min(SBW, NB - b8)
                    zstage = zpool.tile([P, SBW * HID], bf16, tag="zst")
                    for j in range(nb8):
                        b = b8 + j
                        pst = pspool.tile([P, P], f32, tag="ps")
                        nc.tensor.transpose(
                            out=pst[:], in_=hbuf[:, b * P:(b + 1) * P],
                            identity=ident_s[:],
                        )
                        hT_b = wpool.tile([P, P], f32, tag="hTb")
                        nc.vector.tensor_copy(hT_b[:], pst[:])
                        psz = pspool.tile([P, HID], f32, tag="ps")
                        nc.tensor.matmul(
                            psz[:], lhsT=hT_b[:], rhs=Wc_s[:, l, :],
                            start=True, stop=True,
                        )
                        nc.vector.tensor_copy(
                            zstage[:, j * HID:(j + 1) * HID], psz[:]
                        )
                    nc.scalar.activation(
                        zbuf[:, b8 * HID: b8 * HID + nb8 * HID],
                        zstage[:, : nb8 * HID], AFT.Copy,
                    )
                    nc.sync.dma_start(
                        z_local[b8 * P: b8 * P + nb8 * P, :].rearrange(
                            "(b p) h -> p b h", p=P
                        ),
                        zstage[:, : nb8 * HID].rearrange(
                            "p (b h) -> p b h", h=HID
                        ),
                    )
                nc.gpsimd.collective_compute(
                    "AllGather", Alu.bypass,
                    replica_groups=rg,
                    ins=[z_local[:].opt()], outs=[zt[:].opt()],
                )

                for sdef in sched:
                    blocks = sdef["blocks"]
                    aggs = {
                        d: aggpool.tile([P, HID], f32, tag="agg",
                                        name=f"agg_{l}_{d}")
                        for d in blocks
                    }
                    # gather path per sub-table (equal-size chunks: small
                    # dma_gather calls pay a fixed overhead per call)
                    gtile_map = {}
                    for t in range(NTAB):
                        tdef = sdef["tabs"][t]
                        seg = tdef["seg"]
                        ntiles = seg // P
                        nch = -(-seg // GCHUNK)
                        base_t, rem_t = divmod(ntiles, nch)
                        sizes = [
                            (base_t + (i < rem_t)) * P for i in range(nch)
                        ]
                        bounds_ = [0]
                        for n in sizes:
                            bounds_.append(bounds_[-1] + n)
                        tdef["bounds"] = bounds_
                        gtiles = []
                        for ci_, n in enumerate(sizes):
                            off = bounds_[ci_]
                            gt = gpool.tile([P, GCHUNK // P, HID], bf16,
                                            tag="g")
                            s0 = tdef["gslot0"] + off
                            nc.gpsimd.dma_gather(
                                gt[:, : n // P, :],
                                zt[t * TAB_ROWS:(t + 1) * TAB_ROWS, :],
                                idxg_s[:, s0 // 16:(s0 + n) // 16],
                                n, n, HID, single_packet=False,
                            )
                            gtiles.append(gt)
                        gtile_map[t] = gtiles
                        sbt = spool.tile([P, seg], bf16, tag="sg",
                                         name=f"sbt_{l}_{tdef['mcol0']}")
                        nc.vector.tensor_tensor(
                            out=sbt[:].rearrange("p (a k) -> p a k", k=P),
                            in0=dstg_s[
                                :, tdef["mcol0"]:tdef["mcol0"] + seg // P
                            ].to_broadcast([P, seg // P, P]),
                            in1=iota_s[:].rearrange(
                                "p (a k) -> p a k", a=1
                            ).to_broadcast([P, seg // P, P]),
                            op=Alu.is_equal,
                        )
                        tdef["sbt"] = sbt

                    # scatter matmuls
                    import bisect as _bisect
                    for d in blocks:
                        cons = sdef["contrib"][d]
                        for ci, con in enumerate(cons):
                            tdef = sdef["tabs"][con["t"]]
                            srel = con["srel"]
                            bnd = tdef["bounds"]
                            gi_ = _bisect.bisect_right(bnd, srel) - 1
                            col = (srel - bnd[gi_]) // P
                            sb_col = con["mcol"] - tdef["mcol0"]
                            nc.tensor.matmul(
                                aggs[d][:],
                                lhsT=tdef["sbt"][
                                    :, sb_col * P:(sb_col + 1) * P
                                ],
                                rhs=gtile_map[con["t"]][gi_][:, col, :],
                                start=(ci == 0), stop=(ci == len(cons) - 1),
                            )
                    for d in blocks:
                        nc.vector.tensor_copy(
                            hbuf[:, d * P:(d + 1) * P], aggs[d][:]
                        )

                # epilogue: h = tanh(dinv*(agg + z_self) + b); premult dinv
                nc.vector.tensor_tensor(
                    out=hbuf[:].rearrange("p (b h) -> p b h", h=HID),
                    in0=hbuf[:].rearrange("p (b h) -> p b h", h=HID),
                    in1=zbuf[:, :SHARD_PAD].rearrange(
                        "p (b h) -> p b h", h=HID
                    ),
                    op=Alu.add,
                )
                nc.vector.tensor_tensor(
                    out=hbuf[:].rearrange("p (b k) -> p b k", k=P),
                    in0=hbuf[:].rearrange("p (b k) -> p b k", k=P),
                    in1=dinv_s[:].to_broadcast([P, NB, P]),
                    op=Alu.mult,
                )
                nc.vector.tensor_tensor(
                    out=hbuf[:].rearrange("p (b h) -> p b h", h=HID),
                    in0=hbuf[:].rearrange("p (b h) -> p b h", h=HID),
                    in1=bconv_s[:, l, :].rearrange(
                        "p (a h) -> p a h", a=1
                    ).to_broadcast([P, NB, HID]),
                    op=Alu.add,
                )
                nc.scalar.activation(hbuf[:], hbuf[:], AFT.Tanh)
                if l < L - 1:
                    nc.vector.tensor_tensor(
                        out=hbuf[:].rearrange("p (b k) -> p b k", k=P),
                        in0=hbuf[:].rearrange("p (b k) -> p b k", k=P),
                        in1=dinv_s[:].to_broadcast([P, NB, P]),
                        op=Alu.mult,
                    )

            # ---- pooling ----
            hT = bigpool.tile([P, HT_W], bf16, tag="hT")
            if HT_W > SHARD_PAD:
                nc.vector.memset(hT[:, SHARD_PAD:], 0.0)
            for b in range(NB):
                pst = pspool.tile([P, P], f32, tag="ps")
                nc.tensor.transpose(
                    out=pst[:], in_=hbuf[:, b * P:(b + 1) * P],
                    identity=ident_s[:],
                )
                nc.vector.tensor_copy(hT[:, b * P:(b + 1) * P], pst[:])
            gid_s = bigpool.tile([P, ZBW], bf16, tag="zb")
            nc.sync.dma_start(gid_s[:, :HT_W], gid_d[:])

            sumP = wpool.tile([P, gpc], f32, tag="sumP")
            maxP = wpool.tile([P, gpc], f32, tag="maxP")
            for g in range(gpc):
                lo, wd = cfg["lo_u"][g], cfg["wd_u"][g]
                eq = wpool.tile([P, WDMAX], bf16, tag="eq")
                nc.vector.tensor_scalar(
                    eq[:, :wd], gid_s[:, lo:lo + wd], float(g), None,
                    Alu.is_equal,
                )
                msk = wpool.tile([P, WDMAX], f32, tag="msk")
                nc.vector.tensor_tensor(
                    out=msk[:, :wd], in0=hT[:, lo:lo + wd], in1=eq[:, :wd],
                    op=Alu.mult,
                )
                nc.vector.reduce_sum(
                    sumP[:, g:g + 1], msk[:, :wd], axis=mybir.AxisListType.X
                )
                nc.vector.tensor_scalar(
                    msk[:, :wd], eq[:, :wd], 60.0, -60.0, Alu.mult, Alu.add
                )
                nc.vector.tensor_tensor(
                    out=msk[:, :wd], in0=hT[:, lo:lo + wd], in1=msk[:, :wd],
                    op=Alu.add,
                )
                nc.vector.reduce_max(
                    maxP[:, g:g + 1], msk[:, :wd], axis=mybir.AxisListType.X
                )
            pg = pspool.tile([P, 2 * HID], f32, tag="ps")
            nc.tensor.transpose(
                out=pg[:gpc, :HID], in_=sumP[:], identity=ident_s[:]
            )
            nc.tensor.transpose(
                out=pg[:gpc, HID:], in_=maxP[:], identity=ident_s[:]
            )
            pl = wpool.tile([gpc, 2 * HID], f32, tag="pl")
            nc.vector.tensor_copy(pl[:], pg[:gpc, :])
            nc.sync.dma_start(pool_loc[:], pl[:])
            nc.gpsimd.collective_compute(
                "AllGather", Alu.bypass, replica_groups=rg,
                ins=[pool_loc[:].opt()], outs=[pool_all[:].opt()],
            )
            pa = wpool.tile([G, 2 * HID], f32, tag="pa")
            nc.sync.dma_start(pa[:], pool_all[:])
            mfix = wpool.tile([G, HID], f32, tag="mfix")
            nc.vector.tensor_scalar(
                mfix[:], pa[:, HID:], -50.0, None, Alu.is_gt
            )
            nc.vector.tensor_tensor(
                out=pa[:, HID:], in0=pa[:, HID:], in1=mfix[:], op=Alu.mult
            )
            mean_gf = wpool.tile([G, HID], f32, tag="mean")
            nc.vector.tensor_scalar(
                mean_gf[:], pa[:, :HID], cnt_s[:, :1], None, Alu.mult
            )
            gT = wpool.tile([P, 3 * G], f32, tag="gT")
            for a, src_ap in enumerate([pa[:, :HID], pa[:, HID:], mean_gf[:]]):
                ptx = pspool.tile([P, G], f32, tag="ps")
                nc.tensor.transpose(
                    out=ptx[:, :G], in_=src_ap, identity=ident_s[:G, :G]
                )
                nc.vector.tensor_copy(gT[:, a * G:(a + 1) * G], ptx[:, :G])

            # ---- head ----
            ph1 = pspool.tile([G, H1], f32, tag="ps")
            for a in range(3):
                nc.tensor.matmul(
                    ph1[:], lhsT=gT[:, a * G:(a + 1) * G], rhs=W1_s[:, a, :],
                    start=(a == 0), stop=(a == 2),
                )
            g1 = wpool.tile([G, H1], f32, tag="g1")
            nc.vector.tensor_tensor(
                out=g1[:], in0=ph1[:], in1=b1_s[:G, :], op=Alu.add
            )
            nc.scalar.activation(g1[:], g1[:], AFT.Lrelu, alpha=0.01)
            g1T = wpool.tile([H1H, 2 * G], f32, tag="g1T")
            for a in range(2):
                ptt = pspool.tile([H1H, G], f32, tag="ps")
                nc.tensor.transpose(
                    out=ptt[:], in_=g1[:, a * H1H:(a + 1) * H1H],
                    identity=ident_s[:G, :G],
                )
                nc.vector.tensor_copy(g1T[:, a * G:(a + 1) * G], ptt[:])
            ph2 = pspool.tile([G, D_OUT], f32, tag="ps")
            for a in range(2):
                nc.tensor.matmul(
                    ph2[:], lhsT=g1T[:, a * G:(a + 1) * G],
                    rhs=(W2a_s if a == 0 else W2b_s)[:],
                    start=(a == 0), stop=(a == 1),
                )
            go = wpool.tile([G, D_OUT], f32, tag="go")
            nc.vector.tensor_tensor(
                out=go[:], in0=ph2[:], in1=b2_s[:G, :], op=Alu.add
            )
            nc.scalar.activation(go[:], go[:], AFT.Lrelu, alpha=0.01)
            nc.sync.dma_start(out_d[:], go[:])

    nc.compile()
    return nc


def _install_ntff_shim():
    """Provide antenv.axon_hooks (missing in this image) so that
    run_bass_kernel_spmd(trace=True) can capture an NTFF profile via the
    injected libaxon_pjrt.so. Only used when TRACE=True."""
    import types
    import ctypes
    import contextlib

    try:
        from antenv.axon_hooks import get_axon_ntff_profile_hook  # noqa: F401
        return
    except ImportError:
        pass
    so_path = "/opt/axon/libaxon_pjrt.so"
    try:
        lib = ctypes.CDLL(so_path)
    except OSError:
        return
    if not hasattr(lib, "axon_start_nrt_profile"):
        return
    lib.axon_start_nrt_profile.argtypes = [
        ctypes.POINTER(ctypes.c_int64), ctypes.c_size_t,
    ]
    lib.axon_start_nrt_profile.restype = ctypes.c_int64
    lib.axon_stop_nrt_profile.argtypes = [ctypes.c_char_p]
    lib.axon_stop_nrt_profile.restype = ctypes.c_int64

    @contextlib.contextmanager
    def _hook(output_dir, device_ids):
        import jax
        jax.devices()
        if device_ids:
            ids = (ctypes.c_int64 * len(device_ids))(*device_ids)
            rc = lib.axon_start_nrt_profile(ids, len(device_ids))
        else:
            rc = lib.axon_start_nrt_profile(None, 0)
        if rc != 0:
            raise RuntimeError(f"axon_start_nrt_profile rc={rc}")
        try:
            yield
        finally:
            n = lib.axon_stop_nrt_profile(str(output_dir).encode())
            print(f"profile: {n} file(s) written to {output_dir}",
                  file=sys.stderr)

    mod = types.ModuleType("antenv.axon_hooks")
    mod.get_axon_ntff_profile_hook = lambda: _hook
    mod.set_axon_ntff_profile_hook = lambda h: None
    sys.modules["antenv.axon_hooks"] = mod


def kernel(**inputs):
    global LAST_RESULTS
    if TRACE:
        _install_ntff_shim()
    ins = {k: np.asarray(v) for k, v in inputs.items()}
    cfg, common, per_core = _host_prep(
        ins["x"].astype(np.float32), ins["edge_index"], ins["batch"],
        ins["W_emb"], ins["b_emb"], ins["W_conv"], ins["b_conv"],
        ins["W1"], ins["b1"], ins["W2"], ins["b2"],
    )
    nc = _build(cfg)

    in_maps = []
    for c in range(C):
        m = dict(
            xT=per_core[c]["xT"],
            W_emb=common["W_emb"], W_conv=common["W_conv"],
            W1=common["W1"], W2=common["W2"],
            b_emb_b=common["b_emb_b"], b_conv_b=common["b_conv_b"],
            b1_b=common["b1_b"], b2_b=common["b2_b"],
            iota=common["iota"], ident=common["ident"],
            cntinv=common["cntinv"],
            dinv_t=per_core[c]["dinv_t"],
            idxg=per_core[c]["idxg"], dstg=per_core[c]["dstg"],
            gid=per_core[c]["gid"],
        )
        in_maps.append(m)

    res = run_bass_kernel_spmd(
        nc, in_maps, core_ids=list(range(C)), trace=TRACE
    )
    LAST_RESULTS = res
    return np.asarray(res.results[0]["out"], dtype=np.float32)


# revision 7
# speedup vs baseline: 1.2041x; 1.0225x over previous
"""GCN message-passing kernel for Trainium2 (8 NeuronCores).

Strategy (v2):
  - Nodes sharded across 8 cores, aligned to graph boundaries (G/8 graphs/core).
  - Per layer each core computes z = (dinv * h) @ W for its shard, the z-table
    is AllGathered (bf16), source rows are fetched with dma_gather (4 int16
    sub-tables), and one-hot scatter-matmuls accumulate messages per 128-dst
    block in PSUM.
  - Self-loop messages are NOT gathered: agg += z_local directly (SBUF add),
    which cuts gather rows and lowers per-group maxima.
  - Variable per-(block, tab) KD packing minimizes slot padding (the SWDGE
    descriptor generation on GPSIMD at ~8ns/row is the bottleneck engine).
  - Graph pooling: masked static-window reduces on h^T + one AllGather.
"""

import sys

sys.path.insert(0, "/opt/trn_rl_repo")

import numpy as np
import ml_dtypes

import concourse.bass as bass
import concourse.bacc as bacc
import concourse.tile as tile
from concourse import mybir, library_config
from concourse.bass_utils import run_bass_kernel_spmd

C = 8            # cores
P = 128          # partitions / block size
HID = 128
SBW = 4          # dst blocks per superblock
GCHUNK = 3072    # max idxs per dma_gather call

LAST_RESULTS = None  # set by kernel(): BassKernelResults of the last run
TRACE = False        # set True (e.g. by test.py) to capture an NTFF profile


def _host_prep(x, edge_index, batch, W_emb, b_emb, W_conv, b_conv, W1, b1, W2, b2):
    N = x.shape[0]
    batch = np.asarray(batch, dtype=np.int64)
    G = int(batch.max()) + 1
    assert G % C == 0, G
    L = W_conv.shape[0]

    src = np.asarray(edge_index[0], dtype=np.int64)
    dst = np.asarray(edge_index[1], dtype=np.int64)

    # degree includes the self-loop (reference concatenates self edges)
    deg = (np.bincount(dst, minlength=N) + 1).astype(np.float64)
    dinv = (1.0 / np.sqrt(np.maximum(deg, 1e-12))).astype(np.float32)

    gpc = G // C
    bounds = np.searchsorted(batch, np.arange(G + 1))
    starts = bounds[np.arange(C) * gpc]
    ends = bounds[(np.arange(C) + 1) * gpc]
    shard_sizes = ends - starts
    NB = int(np.ceil(shard_sizes.max() / P))
    SHARD_PAD = NB * P
    TAB_ROWS = 2 * SHARD_PAD
    assert TAB_ROWS <= 32767, TAB_ROWS

    core_of = np.searchsorted(ends - 1, np.arange(N), side="left")
    table_row = core_of * SHARD_PAD + (np.arange(N) - starts[core_of])
    e_row = table_row[src]
    e_tab = (e_row // TAB_ROWS).astype(np.int64)
    e_trow = (e_row % TAB_ROWS).astype(np.int64)
    e_core = core_of[dst]
    e_dloc = dst - starts[e_core]
    dblk = e_dloc // P

    NSB = int(np.ceil(NB / SBW))

    NTAB = 4
    cntG = np.zeros((C, NB, NTAB), dtype=np.int64)
    np.add.at(cntG, (e_core, dblk, e_tab), 1)
    KDG = np.maximum(1, np.ceil(cntG.max(axis=0) / P)).astype(np.int64)  # [NB,NTAB]

    # ---- schedule ----
    # gather stream: (sb, t, d, kb); indirect stream: (sb, d, kb)
    sched = []
    gslot = 0   # gather slot offset
    mG = 0      # one-hot column offset (gather)
    for s in range(NSB):
        blocks = list(range(s * SBW, min((s + 1) * SBW, NB)))
        tabs = []
        contrib = {d: [] for d in blocks}
        for t in range(NTAB):
            seg = int(sum(KDG[d, t] for d in blocks)) * P
            tabs.append(dict(seg=seg, mcol0=mG, gslot0=gslot))
            srel = 0
            for d in blocks:
                for kb in range(int(KDG[d, t])):
                    contrib[d].append(dict(t=t, srel=srel, mcol=mG))
                    srel += P
                    mG += 1
            gslot += seg
        sched.append(dict(blocks=blocks, tabs=tabs, contrib=contrib))
    TOT_G = gslot
    NMM_G = mG

    # ---- per-core slot filling ----
    idxG_cores, dstG_cores = [], []
    for c in range(C):
        mc = e_core == c
        key = (dblk[mc] * NTAB + e_tab[mc])
        order = np.argsort(key, kind="stable")
        trow_s = e_trow[mc][order]
        dloc_s = e_dloc[mc][order]
        key_s = key[order]
        first = np.searchsorted(key_s, np.arange(NB * NTAB))
        cnt_k = np.diff(np.append(first, len(key_s)))

        idx_arr = np.zeros(TOT_G, dtype=np.int16)
        dg_arr = np.full((NMM_G, P), -1.0, dtype=np.float32)
        for sdef in sched:
            for t in range(NTAB):
                tdef = sdef["tabs"][t]
                base = tdef["gslot0"]
                srel = 0
                for d in sdef["blocks"]:
                    k = d * NTAB + t
                    n = cnt_k[k]
                    f = first[k]
                    sl = base + srel
                    idx_arr[sl:sl + n] = trow_s[f:f + n].astype(np.int16)
                    mrow = tdef["mcol0"] + (srel // P)
                    dl = (dloc_s[f:f + n] % P).astype(np.float32)
                    kd = int(KDG[d, t])
                    pad = kd * P - n
                    assert pad >= 0
                    dfull = np.concatenate([dl, np.full(pad, -1.0, np.float32)])
                    dg_arr[mrow:mrow + kd] = dfull.reshape(kd, P)
                    srel += kd * P
        idxG_cores.append(np.tile(
            np.ascontiguousarray(idx_arr.reshape(-1, 16).T), (C, 1)
        ))
        dstG_cores.append(np.ascontiguousarray(
            dg_arr.T.astype(ml_dtypes.bfloat16)
        ))

    # ---- pooling windows ----
    lo_u = np.zeros(gpc, dtype=np.int64)
    wd_u = np.zeros(gpc, dtype=np.int64)
    for g in range(gpc):
        los = bounds[np.arange(C) * gpc + g] - starts
        his = bounds[np.arange(C) * gpc + g + 1] - starts
        lo_u[g] = los.min()
        wd_u[g] = max(his.max() - lo_u[g], 1)
    HT_W = int(max(SHARD_PAD, (lo_u + wd_u).max()))

    gid_cores = []
    for c in range(C):
        gid = np.full(HT_W, -1.0, dtype=np.float32)
        n = shard_sizes[c]
        gid[:n] = (batch[starts[c]:ends[c]] - c * gpc).astype(np.float32)
        gid_cores.append(
            np.ascontiguousarray(np.tile(gid.astype(ml_dtypes.bfloat16), (P, 1)))
        )

    cnt = (bounds[1:] - bounds[:-1]).astype(np.float32)
    cntinv = (1.0 / np.maximum(cnt, 1.0)).astype(np.float32)

    xT_cores, dinv_cores = [], []
    D_IN = x.shape[1]
    for c in range(C):
        xs = np.zeros((SHARD_PAD, D_IN), dtype=np.float32)
        xs[: shard_sizes[c]] = x[starts[c]:ends[c]]
        xT_cores.append(np.ascontiguousarray(xs.T))
        dv = np.zeros(SHARD_PAD, dtype=np.float32)
        dv[: shard_sizes[c]] = dinv[starts[c]:ends[c]]
        dinv_cores.append(np.ascontiguousarray(dv.reshape(NB, P).T))

    cfg = dict(
        N=N, G=G, L=L, gpc=gpc, NB=NB, SHARD_PAD=SHARD_PAD, TAB_ROWS=TAB_ROWS,
        NSB=NSB, NTAB=NTAB, TOT_G=TOT_G, NMM_G=NMM_G,
        D_IN=D_IN, HT_W=HT_W, lo_u=lo_u.tolist(), wd_u=wd_u.tolist(),
        D_OUT=W2.shape[1], H1=W1.shape[1], sched=sched,
    )

    common = dict(
        W_emb=np.asarray(W_emb, np.float32),
        W_conv=np.asarray(W_conv, np.float32).reshape(L * HID, HID),
        W1=np.asarray(W1, np.float32),
        W2=np.asarray(W2, np.float32),
        b_emb_b=np.tile(np.asarray(b_emb, np.float32), (P, 1)),
        b_conv_b=np.tile(
            np.asarray(b_conv, np.float32)[:, None, :], (1, P, 1)
        ).reshape(L * P, HID),
        b1_b=np.tile(np.asarray(b1, np.float32), (P, 1)),
        b2_b=np.tile(np.asarray(b2, np.float32), (P, 1)),
        iota=np.tile(np.arange(P, dtype=np.float32), (P, 1)).astype(
            ml_dtypes.bfloat16
        ),
        ident=np.eye(P, dtype=np.float32),
        cntinv=cntinv.reshape(G, 1),
    )
    per_core = [
        dict(
            xT=xT_cores[c], dinv_t=dinv_cores[c],
            idxg=idxG_cores[c], dstg=dstG_cores[c], gid=gid_cores[c],
        )
        for c in range(C)
    ]
    return cfg, common, per_core


def _build(cfg):
    G, L = cfg["G"], cfg["L"]
    gpc, NB, SHARD_PAD = cfg["gpc"], cfg["NB"], cfg["SHARD_PAD"]
    TAB_ROWS, NSB, NTAB = cfg["TAB_ROWS"], cfg["NSB"], cfg["NTAB"]
    TOT_G, NMM_G = cfg["TOT_G"], cfg["NMM_G"]
    D_IN, HT_W = cfg["D_IN"], cfg["HT_W"]
    D_OUT, H1 = cfg["D_OUT"], cfg["H1"]
    sched = cfg["sched"]
    H1H = H1 // 2
    WDMAX = int(max(cfg["wd_u"]))
    f32, bf16 = mybir.dt.float32, mybir.dt.bfloat16
    i16, i32 = mybir.dt.int16, mybir.dt.int32
    AFT = mybir.ActivationFunctionType
    Alu = mybir.AluOpType

    nc = bacc.Bacc("TRN2", target_bir_lowering=False, debug=False, num_devices=C)

    xT = nc.dram_tensor("xT", [D_IN, SHARD_PAD], f32, kind="ExternalInput")
    W_emb = nc.dram_tensor("W_emb", [D_IN, HID], f32, kind="ExternalInput")
    W_conv = nc.dram_tensor("W_conv", [L * HID, HID], f32, kind="ExternalInput")
    W1 = nc.dram_tensor("W1", [3 * HID, H1], f32, kind="ExternalInput")
    W2 = nc.dram_tensor("W2", [H1, D_OUT], f32, kind="ExternalInput")
    b_emb_b = nc.dram_tensor("b_emb_b", [P, HID], f32, kind="ExternalInput")
    b_conv_b = nc.dram_tensor("b_conv_b", [L * P, HID], f32, kind="ExternalInput")
    b1_b = nc.dram_tensor("b1_b", [P, H1], f32, kind="ExternalInput")
    b2_b = nc.dram_tensor("b2_b", [P, D_OUT], f32, kind="ExternalInput")
    iota_d = nc.dram_tensor("iota", [P, P], bf16, kind="ExternalInput")
    ident_d = nc.dram_tensor("ident", [P, P], f32, kind="ExternalInput")
    cntinv_d = nc.dram_tensor("cntinv", [G, 1], f32, kind="ExternalInput")
    dinv_d = nc.dram_tensor("dinv_t", [P, NB], f32, kind="ExternalInput")
    idxg_d = nc.dram_tensor("idxg", [P, TOT_G // 16], i16, kind="ExternalInput")
    dstg_d = nc.dram_tensor("dstg", [P, NMM_G], bf16, kind="ExternalInput")
    gid_d = nc.dram_tensor("gid", [P, HT_W], bf16, kind="ExternalInput")
    out_d = nc.dram_tensor("out", [G, D_OUT], f32, kind="ExternalOutput")

    z_local = nc.dram_tensor("z_local", [SHARD_PAD, HID], bf16, kind="Internal")
    z_tables = [
        nc.dram_tensor(f"z_table{i}", [C * SHARD_PAD, HID], bf16, kind="Internal")
        for i in range(2)
    ]
    pool_loc = nc.dram_tensor("pool_loc", [gpc, 2 * HID], f32, kind="Internal")
    pool_all = nc.dram_tensor("pool_all", [G, 2 * HID], f32, kind="Internal")

    rg = [list(range(C))]
    ZBW = max(SHARD_PAD, HT_W)

    with tile.TileContext(nc) as tc:
        with (
            tc.tile_pool(name="const", bufs=1) as cpool,
            tc.tile_pool(name="big", bufs=1) as bigpool,
            tc.tile_pool(name="g", bufs=3) as gpool,
            tc.tile_pool(name="s", bufs=2) as spool,
            tc.tile_pool(name="work", bufs=2) as wpool,
            tc.tile_pool(name="zst", bufs=2) as zpool,
            tc.tile_pool(name="ps", bufs=3, space="PSUM") as pspool,
            tc.tile_pool(name="agg", bufs=4, space="PSUM") as aggpool,
        ):
            nc.gpsimd.load_library(library_config.mlp)

            def cload(dram_ap, shape, dtype, nm):
                t = cpool.tile(shape, dtype, name=nm, tag=nm)
                nc.sync.dma_start(t[:], dram_ap)
                return t

            Wemb_s = cload(W_emb[:], [D_IN, HID], f32, "Wemb_s")
            Wc_s = cload(
                W_conv[:].rearrange("(l k) h -> k l h", k=P), [P, L, HID], f32
            , "Wc_s")
            W1_s = cload(W1[:].rearrange("(a k) h -> k a h", k=P), [P, 3, H1], f32, "W1_s")
            W2a_s = cload(W2[0:H1H, :], [H1H, D_OUT], f32, "W2a_s")
            W2b_s = cload(W2[H1H:H1, :], [H1H, D_OUT], f32, "W2b_s")
            bemb_s = cload(b_emb_b[:], [P, HID], f32, "bemb_s")
            bconv_s = cload(
                b_conv_b[:].rearrange("(l k) h -> k l h", k=P), [P, L, HID], f32
            , "bconv_s")
            b1_s = cload(b1_b[:], [P, H1], f32, "b1_s")
            b2_s = cload(b2_b[:], [P, D_OUT], f32, "b2_s")
            iota_s = cload(iota_d[:], [P, P], bf16, "iota_s")
            ident_s = cload(ident_d[:], [P, P], f32, "ident_s")
            cnt_s = cload(cntinv_d[:], [G, 1], f32, "cnt_s")
            dinv_s = cload(dinv_d[:], [P, NB], f32, "dinv_s")
            dstg_s = cload(dstg_d[:], [P, NMM_G], bf16, "dstg_s")
            idxg_s = cload(idxg_d[:], [P, TOT_G // 16], i16, "idxg_s")

            hbuf = bigpool.tile([P, SHARD_PAD], f32, tag="h")
            zbuf = bigpool.tile([P, ZBW], bf16, tag="zb")

            # ---- embed: h'0 = dinv * (x @ W_emb + b_emb) ----
            for b4 in range(0, NB, 4):
                nb4 = min(4, NB - b4)
                ps = pspool.tile([P, 4 * HID], f32, tag="ps")
                for j in range(nb4):
                    b = b4 + j
                    xt_b = wpool.tile([D_IN, P], f32, tag="xt")
                    nc.sync.dma_start(xt_b[:], xT[:, b * P:(b + 1) * P])
                    nc.tensor.matmul(
                        ps[:, j * HID:(j + 1) * HID],
                        lhsT=xt_b[:], rhs=Wemb_s[:],
                        start=True, stop=True,
                    )
                nc.vector.tensor_copy(
                    hbuf[:, b4 * P: b4 * P + nb4 * HID], ps[:, : nb4 * HID]
                )
            nc.vector.tensor_tensor(
                out=hbuf[:].rearrange("p (b h) -> p b h", h=HID),
                in0=hbuf[:].rearrange("p (b h) -> p b h", h=HID),
                in1=bemb_s[:].rearrange("p (a h) -> p a h", a=1).to_broadcast(
                    [P, NB, HID]
                ),
                op=Alu.add,
            )
            nc.vector.tensor_tensor(
                out=hbuf[:].rearrange("p (b k) -> p b k", k=P),
                in0=hbuf[:].rearrange("p (b k) -> p b k", k=P),
                in1=dinv_s[:].to_broadcast([P, NB, P]),
                op=Alu.mult,
            )

            # ---- layers ----
            for l in range(L):
                zt = z_tables[l % 2]
                # z compute (node-major) + zbuf copy + z_local write
                for b8 in range(0, NB, SBW):
                    nb8 = min(SBW, NB - b8)
                    zstage = zpool.tile([P, SBW * HID], bf16, tag="zst")
                    for j in range(nb8):
                        b = b8 + j
                        pst = pspool.tile([P, P], f32, tag="ps")
                        nc.tensor.transpose(
                            out=pst[:], in_=hbuf[:, b * P:(b + 1) * P],
                            identity=ident_s[:],
                        )
                        hT_b = wpool.tile([P, P], f32, tag="hTb")
                        nc.vector.tensor_copy(hT_b[:], pst[:])
                        psz = pspool.tile([P, HID], f32, tag="ps")
                        nc.tensor.matmul(
                            psz[:], lhsT=hT_b[:], rhs=Wc_s[:, l, :],
                            start=True, stop=True,
                        )
                        nc.vector.tensor_copy(
                            zstage[:, j * HID:(j + 1) * HID], psz[:]
                        )
                    nc.scalar.activation(
                        zbuf[:, b8 * HID: b8 * HID + nb8 * HID],
                        zstage[:, : nb8 * HID], AFT.Copy,
                    )
                    nc.sync.dma_start(
                        z_local[b8 * P: b8 * P + nb8 * P, :].rearrange(
                            "(b p) h -> p b h", p=P
                        ),
                        zstage[:, : nb8 * HID].rearrange(
                            "p (b h) -> p b h", h=HID
                        ),
                    )
                nc.gpsimd.collective_compute(
                    "AllGather", Alu.bypass,
                    replica_groups=rg,
                    ins=[z_local[:].opt()], outs=[zt[:].opt()],
                )

                for sdef in sched:
                    blocks = sdef["blocks"]
                    aggs = {
                        d: aggpool.tile([P, HID], f32, tag="agg",
                                        name=f"agg_{l}_{d}")
                        for d in blocks
                    }
                    # gather path per sub-table (equal-size chunks: small
                    # dma_gather calls pay a fixed overhead per call)
                    gtile_map = {}
                    for t in range(NTAB):
                        tdef = sdef["tabs"][t]
                        seg = tdef["seg"]
                        ntiles = seg // P
                        nch = -(-seg // GCHUNK)
                        base_t, rem_t = divmod(ntiles, nch)
                        sizes = [
                            (base_t + (i < rem_t)) * P for i in range(nch)
                        ]
                        bounds_ = [0]
                        for n in sizes:
                            bounds_.append(bounds_[-1] + n)
                        tdef["bounds"] = bounds_
                        gtiles = []
                        for ci_, n in enumerate(sizes):
                            off = bounds_[ci_]
                            gt = gpool.tile([P, GCHUNK // P, HID], bf16,
                                            tag="g")
                            s0 = tdef["gslot0"] + off
                            nc.gpsimd.dma_gather(
                                gt[:, : n // P, :],
                                zt[t * TAB_ROWS:(t + 1) * TAB_ROWS, :],
                                idxg_s[:, s0 // 16:(s0 + n) // 16],
                                n, n, HID, single_packet=False,
                            )
                            gtiles.append(gt)
                        gtile_map[t] = gtiles
                        sbt = spool.tile([P, seg], bf16, tag="sg",
                                         name=f"sbt_{l}_{tdef['mcol0']}")
                        nc.vector.tensor_tensor(
                            out=sbt[:].rearrange("p (a k) -> p a k", k=P),
                            in0=dstg_s[
                                :, tdef["mcol0"]:tdef["mcol0"] + seg // P
                            ].to_broadcast([P, seg // P, P]),
                            in1=iota_s[:].rearrange(
                                "p (a k) -> p a k", a=1
                            ).to_broadcast([P, seg // P, P]),
                            op=Alu.is_equal,
                        )
                        tdef["sbt"] = sbt

                    # scatter matmuls
                    import bisect as _bisect
                    for d in blocks:
                        cons = sdef["contrib"][d]
                        for ci, con in enumerate(cons):
                            tdef = sdef["tabs"][con["t"]]
                            srel = con["srel"]
                            bnd = tdef["bounds"]
                            gi_ = _bisect.bisect_right(bnd, srel) - 1
                            col = (srel - bnd[gi_]) // P
                            sb_col = con["mcol"] - tdef["mcol0"]
                            nc.tensor.matmul(
                                aggs[d][:],
                                lhsT=tdef["sbt"][
                                    :, sb_col * P:(sb_col + 1) * P
                                ],
                                rhs=gtile_map[con["t"]][gi_][:, col, :],
                                start=(ci == 0), stop=(ci == len(cons) - 1),
                            )
                    for d in blocks:
                        nc.vector.tensor_copy(
                            hbuf[:, d * P:(d + 1) * P], aggs[d][:]
                        )

                # epilogue: h = tanh(dinv*(agg + z_self) + b); premult dinv
                nc.vector.tensor_tensor(
                    out=hbuf[:].rearrange("p (b h) -> p b h", h=HID),
                    in0=hbuf[:].rearrange("p (b h) -> p b h", h=HID),
                    in1=zbuf[:, :SHARD_PAD].rearrange(
                        "p (b h) -> p b h", h=HID
                    ),
                    op=Alu.add,
                )
                nc.vector.tensor_tensor(
                    out=hbuf[:].rearrange("p (b k) -> p b k", k=P),
                    in0=hbuf[:].rearrange("p (b k) -> p b k", k=P),
                    in1=dinv_s[:].to_broadcast([P, NB, P]),
                    op=Alu.mult,
                )
                nc.vector.tensor_tensor(
                    out=hbuf[:].rearrange("p (b h) -> p b h", h=HID),
                    in0=hbuf[:].rearrange("p (b h) -> p b h", h=HID),
                    in1=bconv_s[:, l, :].rearrange(
                        "p (a h) -> p a h", a=1
                    ).to_broadcast([P, NB, HID]),
                    op=Alu.add,
                )
                nc.scalar.activation(hbuf[:], hbuf[:], AFT.Tanh)
                if l < L - 1:
                    nc.vector.tensor_tensor(
                        out=hbuf[:].rearrange("p (b k) -> p b k", k=P),
                        in0=hbuf[:].rearrange("p (b k) -> p b k", k=P),
                        in1=dinv_s[:].to_broadcast([P, NB, P]),
                        op=Alu.mult,
                    )

            # ---- pooling ----
            hT = bigpool.tile([P, HT_W], bf16, tag="hT")
            if HT_W > SHARD_PAD:
                nc.vector.memset(hT[:, SHARD_PAD:], 0.0)
            for b in range(NB):
                pst = pspool.tile([P, P], f32, tag="ps")
                nc.tensor.transpose(
                    out=pst[:], in_=hbuf[:, b * P:(b + 1) * P],
                    identity=ident_s[:],
                )
                nc.vector.tensor_copy(hT[:, b * P:(b + 1) * P], pst[:])
            gid_s = bigpool.tile([P, ZBW], bf16, tag="zb")
            nc.sync.dma_start(gid_s[:, :HT_W], gid_d[:])

            sumP = wpool.tile([P, gpc], f32, tag="sumP")
            maxP = wpool.tile([P, gpc], f32, tag="maxP")
            for g in range(gpc):
                lo, wd = cfg["lo_u"][g], cfg["wd_u"][g]
                eq = wpool.tile([P, WDMAX], bf16, tag="eq")
                nc.vector.tensor_scalar(
                    eq[:, :wd], gid_s[:, lo:lo + wd], float(g), None,
                    Alu.is_equal,
                )
                msk = wpool.tile([P, WDMAX], f32, tag="msk")
                nc.vector.tensor_tensor(
                    out=msk[:, :wd], in0=hT[:, lo:lo + wd], in1=eq[:, :wd],
                    op=Alu.mult,
                )
                nc.vector.reduce_sum(
                    sumP[:, g:g + 1], msk[:, :wd], axis=mybir.AxisListType.X
                )
                nc.vector.tensor_scalar(
                    msk[:, :wd], eq[:, :wd], 60.0, -60.0, Alu.mult, Alu.add
                )
                nc.vector.tensor_tensor(
                    out=msk[:, :wd], in0=hT[:, lo:lo + wd], in1=msk[:, :wd],
                    op=Alu.add,
                )
                nc.vector.reduce_max(
                    maxP[:, g:g + 1], msk[:, :wd], axis=mybir.AxisListType.X
                )
            pg = pspool.tile([P, 2 * HID], f32, tag="ps")
            nc.tensor.transpose(
                out=pg[:gpc, :HID], in_=sumP[:], identity=ident_s[:]
            )
            nc.tensor.transpose(
                out=pg[:gpc, HID:], in_=maxP[:], identity=ident_s[:]
            )
            pl = wpool.tile([gpc, 2 * HID], f32, tag="pl")
            nc.vector.tensor_copy(pl[:], pg[:gpc, :])
            nc.sync.dma_start(pool_loc[:], pl[:])
            nc.gpsimd.collective_compute(
                "AllGather", Alu.bypass, replica_groups=rg,
                ins=[pool_loc[:].opt()], outs=[pool_all[:].opt()],
            )
            pa = wpool.tile([G, 2 * HID], f32, tag="pa")
            nc.sync.dma_start(pa[:], pool_all[:])
            mfix = wpool.tile([G, HID], f32, tag="mfix")
            nc.vector.tensor_scalar(
                mfix[:], pa[:, HID:], -50.0, None, Alu.is_gt
            )
            nc.vector.tensor_tensor(
                out=pa[:, HID:], in0=pa[:, HID:], in1=mfix[:], op=Alu.mult
            )
            mean_gf = wpool.tile([G, HID], f32, tag="mean")
            nc.vector.tensor_scalar(
                mean_gf[:], pa[:, :HID], cnt_s[:, :1], None, Alu.mult
            )
            gT = wpool.tile([P, 3 * G], f32, tag="gT")
            for a, src_ap in enumerate([pa[:, :HID], pa[:, HID:], mean_gf[:]]):
                ptx = pspool.tile([P, G], f32, tag="ps")
                nc.tensor.transpose(
                    out=ptx[:, :G], in_=src_ap, identity=ident_s[:G, :G]
                )
                nc.vector.tensor_copy(gT[:, a * G:(a + 1) * G], ptx[:, :G])

            # ---- head ----
            ph1 = pspool.tile([G, H1], f32, tag="ps")
            for a in range(3):
                nc.tensor.matmul(
                    ph1[:], lhsT=gT[:, a * G:(a + 1) * G], rhs=W1_s[:, a, :],
                    start=(a == 0), stop=(a == 2),
                )
            g1 = wpool.tile([G, H1], f32, tag="g1")
            nc.vector.tensor_tensor(
                out=g1[:], in0=ph1[:], in1=b1_s[:G, :], op=Alu.add
            )
            nc.scalar.activation(g1[:], g1[:], AFT.Lrelu, alpha=0.01)
            g1T = wpool.tile([H1H, 2 * G], f32, tag="g1T")
            for a in range(2):
                ptt = pspool.tile([H1H, G], f32, tag="ps")
                nc.tensor.transpose(
                    out=ptt[:], in_=g1[:, a * H1H:(a + 1) * H1H],
                    identity=ident_s[:G, :G],
                )
                nc.vector.tensor_copy(g1T[:, a * G:(a + 1) * G], ptt[:])
            ph2 = pspool.tile([G, D_OUT], f32, tag="ps")
            for a in range(2):
                nc.tensor.matmul(
                    ph2[:], lhsT=g1T[:, a * G:(a + 1) * G],
                    rhs=(W2a_s if a == 0 else W2b_s)[:],
                    start=(a == 0), stop=(a == 1),
                )
            go = wpool.tile([G, D_OUT], f32, tag="go")
            nc.vector.tensor_tensor(
                out=go[:], in0=ph2[:], in1=b2_s[:G, :], op=Alu.add
            )
            nc.scalar.activation(go[:], go[:], AFT.Lrelu, alpha=0.01)
            nc.sync.dma_start(out_d[:], go[:])

    nc.compile()
    return nc


def _install_ntff_shim():
    """Provide antenv.axon_hooks (missing in this image) so that
    run_bass_kernel_spmd(trace=True) can capture an NTFF profile via the
    injected libaxon_pjrt.so. Only used when TRACE=True."""
    import types
    import ctypes
    import contextlib

    try:
        from antenv.axon_hooks import get_axon_ntff_profile_hook  # noqa: F401
        return
    except ImportError:
        pass
    so_path = "/opt/axon/libaxon_pjrt.so"
    try:
        lib = ctypes.CDLL(so_path)
    except OSError:
        return
    if not hasattr(lib, "axon_start_nrt_profile"):
        return
    lib.axon_start_nrt_profile.argtypes = [
        ctypes.POINTER(ctypes.c_int64), ctypes.c_size_t,
    ]
    lib.axon_start_nrt_profile.restype = ctypes.c_int64
    lib.axon_stop_nrt_profile.argtypes = [ctypes.c_char_p]
    lib.axon_stop_nrt_profile.restype = ctypes.c_int64

    @contextlib.contextmanager
    def _hook(output_dir, device_ids):
        import jax
        jax.devices()
        if device_ids:
            ids = (ctypes.c_int64 * len(device_ids))(*device_ids)
            rc = lib.axon_start_nrt_profile(ids, len(device_ids))
        else:
            rc = lib.axon_start_nrt_profile(None, 0)
        if rc != 0:
            raise RuntimeError(f"axon_start_nrt_profile rc={rc}")
        try:
            yield
        finally:
            n = lib.axon_stop_nrt_profile(str(output_dir).encode())
            print(f"profile: {n} file(s) written to {output_dir}",
                  file=sys.stderr)

    mod = types.ModuleType("antenv.axon_hooks")
    mod.get_axon_ntff_profile_hook = lambda: _hook
    mod.set_axon_ntff_profile_hook = lambda h: None
    sys.modules["antenv.axon_hooks"] = mod


def kernel(**inputs):
    global LAST_RESULTS
    if TRACE:
        _install_ntff_shim()
    ins = {k: np.asarray(v) for k, v in inputs.items()}
    cfg, common, per_core = _host_prep(
        ins["x"].astype(np.float32), ins["edge_index"], ins["batch"],
        ins["W_emb"], ins["b_emb"], ins["W_conv"], ins["b_conv"],
        ins["W1"], ins["b1"], ins["W2"], ins["b2"],
    )
    nc = _build(cfg)

    in_maps = []
    for c in range(C):
        m = dict(
            xT=per_core[c]["xT"],
            W_emb=common["W_emb"], W_conv=common["W_conv"],
            W1=common["W1"], W2=common["W2"],
            b_emb_b=common["b_emb_b"], b_conv_b=common["b_conv_b"],
            b1_b=common["b1_b"], b2_b=common["b2_b"],
            iota=common["iota"], ident=common["ident"],
            cntinv=common["cntinv"],
            dinv_t=per_core[c]["dinv_t"],
            idxg=per_core[c]["idxg"], dstg=per_core[c]["dstg"],
            gid=per_core[c]["gid"],
        )
        in_maps.append(m)

    res = run_bass_kernel_spmd(
        nc, in_maps, core_ids=list(range(C)), trace=TRACE
    )
    LAST_RESULTS = res
    return np.asarray(res.results[0]["out"], dtype=np.float32)
